# revision 29
# baseline (speedup 1.0000x reference)
"""CrossAttentionBlockLLaMA on 8 Trainium2 NeuronCores (Bass/Tile).

Sharding (unchanged from baseline):
  - QKV + attention: tensor-parallel over heads (2 heads/core).
  - wo: row-sharded; per-core partial h for ALL tokens written window-major
    [8, D, TC]; ReduceScatter sums partials -> core r owns tokens_r.
  - FFN + post-norm: token-parallel (TC tokens/core), full weights.

Perf changes vs baseline:
  - fp8 e4m3 DoubleRow matmuls for QKV / wo / FFN (1.44x PE), scores in
    fp8 normal mode, PV/sum in fp16. Power-of-2 static scales keep all
    tensors centered in e4m3 range; final output path stays fp32.
    (out ~= x + ff with ||ff|| ~ 4e-4*||x||, so the fp8 noise on the
    attention+FFN path is attenuated ~2000x in the output.)
  - All row->128-partition broadcasts done with K=1 PE matmuls instead of
    DRAM bounce DMAs (the bounces deadlocked behind the ReduceScatter).
  - rsqrt/silu/reciprocal folded into single scalar ACTs.
  - Host pre-swizzles every weight/activation into its SBUF layout so all
    DMA loads are contiguous; weight streams ride the gpsimd queue.
  - Phase-3(x) inputs prefetched before ReduceScatter(y) is triggered.

Self-contained: hardcodes shapes from the problem spec.
"""
import numpy as np

NCORES = 8
EPS = 1e-5

# power-of-2 static scales (inputs are randn; generous sigma margins)
S_X = 2.0 ** 4     # x,y activations
S_WQ = 2.0 ** 14   # folded wq (incl attn_norm_w and 1/sqrt(HD))
S_WK = 2.0 ** 10
S_WV = 2.0 ** 10
S_Q = 2.0 ** 8     # q8 scratch
S_K = 2.0 ** 5     # k8 scratch
S_V = 2.0 ** 6     # v16 scratch carries rsq*2^6
S_O = 2.0 ** 7     # o8 (normalized attention out)
S_WO = 2.0 ** 10
S_H = 2.0 ** 4     # h after ReduceScatter (folded into hp copy)
S_W1 = 2.0 ** 9
S_W3 = 2.0 ** 8
S_W2 = 2.0 ** 9
S_ZG = S_W3 * S_H          # 2^13, implicit scale of zg8
S_FF = S_W2 * S_ZG         # 2^22, scale of ffn psum and residual

C_Q = S_Q / (S_WQ * S_X)   # 2^-10  (replicate col for q)
C_K = S_K / (S_WK * S_X)   # 2^-9
C_V = S_V / (S_WV * S_X)   # 2^-8   (folded into rsqv row)
C_SO = S_O / S_V           # 2^3    (replicate col for o normalize)
EXP_SCALE = 1.0 / (S_Q * S_K)      # 2^-13
HP_SCALE = S_H / (S_WO * S_O)      # 2^-15
SILU_SCALE = 1.0 / (S_W1 * S_H)    # 2^-13


class Cfg:
    def __init__(self, B=2, S=2048, D=2048, H=16, HD=128, FF=5632):
        self.B, self.S, self.D, self.H, self.HD, self.FF = B, S, D, H, HD, FF
        self.T = B * S                    # total tokens
        self.TC = self.T // NCORES        # tokens per core (phase 3)
        self.NQ = (H // NCORES) * HD      # per-core head dims
        self.DT = D // 128                # d-tiles
        self.FT = FF // 128               # ff-tiles
        self.NQT = self.NQ // 128         # per-core head-dim tiles
        self.TCH = min(512, self.T)       # phase-1 token chunk
        self.QCH = min(512, S)            # phase-2 query chunk
        self.TCW = min(512, self.TC)      # phase-3 / wo token chunk
        self.FB = self.FT // 4            # w1/w3 column groups (512 wide)
        self.DB = self.DT // 2            # w2 column groups (256 wide)
        assert self.T % self.TCH == 0 and S % self.QCH == 0
        assert self.TC % self.TCW == 0 and S % 128 == 0
        assert HD == 128 and D % 128 == 0 and FF % 128 == 0
        assert self.DT % 2 == 0 and self.FT % 4 == 0 and self.NQT == 2


FULL = Cfg()


def build(cfg=FULL):
    import concourse.mybir as mybir
    import concourse.tile as tile
    from concourse import bacc

    F8 = mybir.dt.float8e4
    F16 = mybir.dt.float16
    F32 = mybir.dt.float32

    c = cfg
    nc = bacc.Bacc("TRN2", target_bir_lowering=False, debug=False,
                   num_devices=NCORES)

    ins = {}
    outs = {}
    for s in ("x", "y"):
        # activations pre-swizzled [p, o, t], *S_X, fp8
        ins[f"{s}T8"] = nc.dram_tensor(f"{s}T8", [128, c.DT, c.T], F8,
                                       kind="ExternalInput").ap()
        for w in ("wq", "wk", "wv"):
            ins[f"{w}8_{s}"] = nc.dram_tensor(
                f"{w}8_{s}", [128, c.DT, c.NQ], F8, kind="ExternalInput").ap()
        ins[f"wo8_{s}"] = nc.dram_tensor(
            f"wo8_{s}", [128, c.NQT, c.D], F8, kind="ExternalInput").ap()
        ins[f"w18_{s}"] = nc.dram_tensor(
            f"w18_{s}", [128, c.FB, c.DT * 512], F8, kind="ExternalInput").ap()
        ins[f"w38_{s}"] = nc.dram_tensor(
            f"w38_{s}", [128, c.FB, c.DT * 512], F8, kind="ExternalInput").ap()
        ins[f"w28_{s}"] = nc.dram_tensor(
            f"w28_{s}", [128, c.DB, c.FT * 256], F8, kind="ExternalInput").ap()
        ins[f"res_{s}"] = nc.dram_tensor(
            f"res_{s}", [c.D, c.TC], F32, kind="ExternalInput").ap()
        ins[f"fnorm_{s}"] = nc.dram_tensor(
            f"fnorm_{s}", [128, c.DT], F32, kind="ExternalInput").ap()
        outs[s] = nc.dram_tensor(f"out_{s}", [c.D, c.TC], F32,
                                 kind="ExternalOutput").ap()

    with tile.TileContext(nc) as tc:
        _emit(tc, nc, c, ins, outs)
    nc.compile()
    return nc


def _emit(tc, nc, c, ins, outs):
    import concourse.mybir as mybir

    F8 = mybir.dt.float8e4
    F16 = mybir.dt.float16
    BF16 = mybir.dt.bfloat16
    F32 = mybir.dt.float32
    AF = mybir.ActivationFunctionType
    DR = mybir.MatmulPerfMode.DoubleRow

    with (
        tc.tile_pool(name="psum", bufs=1, space="PSUM") as ps,
        tc.tile_pool(name="const", bufs=1) as const,
        tc.tile_pool(name="dram", bufs=1, space="DRAM") as dram,
    ):
        ones_col = const.tile([128, 1], F16)
        nc.vector.memset(ones_col[:], 1.0)
        onesb_col = const.tile([128, 1], BF16)
        nc.vector.memset(onesb_col[:], 1.0)
        col_cq = const.tile([1, 128], F16)
        nc.vector.memset(col_cq[:], C_Q)
        col_ck = const.tile([1, 128], F16)
        nc.vector.memset(col_ck[:], C_K)
        col_so = const.tile([1, 128], F16)
        nc.vector.memset(col_so[:], C_SO)
        col_fn = const.tile([1, 128], F32)
        nc.vector.memset(col_fn[:], 1.0 / S_FF)
        hp_col = const.tile([128, 1], F32)
        nc.vector.memset(hp_col[:], HP_SCALE)
        eps1 = const.tile([1, 1], F32)
        nc.vector.memset(eps1[:], EPS)

        sc = {}
        for s in ("x", "y"):
            sc[f"q8_{s}"] = dram.tile([c.NQ, c.T], F8, name=f"q8_{s}")
            sc[f"k8_{s}"] = dram.tile([c.NQ, c.T], F8, name=f"k8_{s}")
            # v16: [token%128, token-block, per-core feature]
            sc[f"v16_{s}"] = dram.tile([128, c.T // 128, c.NQ], F16,
                                       name=f"v16_{s}")

            sc[f"hp_{s}"] = dram.tile([NCORES * c.D, c.TC], F16,
                                      name=f"hp_{s}")
            sc[f"h_{s}"] = dram.tile([c.D, c.TC], F16, name=f"h_{s}")
            sc[f"rvd_{s}"] = dram.tile([1, c.TCH], F32, tag=f"rvd_{s}",
                                       bufs=2, name=f"rvd_{s}")

        def mm(shape, name):
            return ps.tile(shape, F32, tag="mm", bufs=4, name=name)

        def row(shape, name):
            return ps.tile(shape, F32, tag="row", bufs=2, name=name)

        def bc(name, dtype=F32):
            return ps.tile([128, 512], dtype, tag="bc", bufs=2, name=name)

        # ============ PHASE 1: RMSNorm stats + QKV projections =============
        with (
            tc.tile_pool(name="p1w", bufs=1) as p1w,
            tc.tile_pool(name="p1a", bufs=2) as p1a,
            tc.tile_pool(name="p1s", bufs=3) as p1s,
        ):
            W = {}
            for s in ("x", "y"):
                for w in ("wq", "wk", "wv"):
                    t = p1w.tile([128, c.DT, c.NQ], F8, name=f"{w}_{s}_sb")
                    nc.gpsimd.dma_start(t[:], ins[f"{w}8_{s}"][:])
                    W[f"{w}{s}"] = t

            for ich in range(c.T // c.TCH):
                tsl = slice(ich * c.TCH, (ich + 1) * c.TCH)
                act = {}
                rsq16 = {}
                rsqvT = {}
                for s in ("x", "y"):
                    at = p1a.tile([128, c.DT, c.TCH], F8, tag=f"act_{s}",
                                  name=f"act_{s}")
                    nc.sync.dma_start(at[:], ins[f"{s}T8"][:, :, tsl])
                    act[s] = at

                    ms_ps = row([1, c.TCH], f"ms_{s}")
                    for o in range(c.DT):
                        sq = p1s.tile([128, c.TCH], F16, tag="sq",
                                      name=f"sq_{s}{o}")
                        eng = nc.vector if o % 2 == 0 else nc.gpsimd
                        eng.tensor_mul(sq[:], at[:, o], at[:, o])
                        nc.tensor.matmul(ms_ps[:], ones_col[:], sq[:],
                                         start=(o == 0), stop=(o == c.DT - 1))
                    # rms -> 1/rms (fast approx) ; rsqv = C_V * rsq
                    rms32 = p1s.tile([1, c.TCH], F32, tag="rms",
                                     name=f"rms_{s}")
                    nc.scalar.activation(rms32[:], ms_ps[:], AF.Sqrt,
                                         bias=eps1[:],
                                         scale=1.0 / (c.D * S_X * S_X))
                    rsq32 = p1s.tile([1, c.TCH], F32, tag="rsq32",
                                     name=f"rq32_{s}")
                    nc.vector.reciprocal_approx_fast(rsq32[:], rms32[:])
                    r16 = p1s.tile([1, c.TCH], F16, tag="rsq", name=f"rq_{s}")
                    nc.vector.tensor_copy(r16[:], rsq32[:])
                    rsq16[s] = r16
                    rv32 = p1s.tile([1, c.TCH], F32, tag="rsqv",
                                    name=f"rv_{s}")
                    nc.scalar.activation(rv32[:], rsq32[:], AF.Copy,
                                         scale=C_V)
                    # bounce rsqv row -> [128, nsub] per-partition scalars
                    nc.sync.dma_start(sc[f"rvd_{s}"][:], rv32[:])
                    rT = p1s.tile([128, c.TCH // 128], F32, tag="rsqT",
                                  name=f"rT_{s}")
                    nc.sync.dma_start(
                        rT[:],
                        sc[f"rvd_{s}"][0, :].rearrange("(n p) -> p n", p=128))
                    rsqvT[s] = rT

                for s in ("x", "y"):
                    kv = "y" if s == "x" else "x"
                    bq_ps = bc(f"bq_{s}")
                    nc.tensor.matmul(bq_ps[:], col_cq[:], rsq16[s][:],
                                     start=True, stop=True)
                    bq16 = p1s.tile([128, c.TCH], F16, tag="bq16",
                                    name=f"bq16_{s}")
                    nc.vector.tensor_copy(bq16[:], bq_ps[:])
                    bk_ps = bc(f"bk_{s}")
                    nc.tensor.matmul(bk_ps[:], col_ck[:], rsq16[kv][:],
                                     start=True, stop=True)
                    bk16 = p1s.tile([128, c.TCH], F16, tag="bk16",
                                    name=f"bk16_{s}")
                    nc.vector.tensor_copy(bk16[:], bk_ps[:])

                    for (wname, src, bct, dst) in (
                        ("wq", s, bq16, sc[f"q8_{s}"]),
                        ("wk", kv, bk16, sc[f"k8_{s}"]),
                    ):
                        wt = W[f"{wname}{s}"]
                        for jt in range(c.NQT):
                            pm = mm([128, c.TCH], f"{wname}{s}{jt}")
                            for o in range(c.DT // 2):
                                nc.tensor.matmul(
                                    pm[:],
                                    wt[:, 2 * o:2 * o + 2,
                                       jt * 128:(jt + 1) * 128],
                                    act[src][:, 2 * o:2 * o + 2, :],
                                    start=(o == 0), stop=(o == c.DT // 2 - 1),
                                    perf_mode=DR)
                            ot8 = p1s.tile([128, c.TCH], F8, tag="proj_out",
                                           name=f"{wname}{s}{jt}o")
                            nc.vector.tensor_mul(ot8[:], pm[:], bct[:])
                            nc.sync.dma_start(
                                dst[jt * 128:(jt + 1) * 128, tsl], ot8[:])

                    for i in range(c.TCH // 128):
                        pv = mm([128, c.NQ], f"v{s}{i}")
                        for o in range(c.DT // 2):
                            nc.tensor.matmul(
                                pv[:],
                                act[kv][:, 2 * o:2 * o + 2,
                                        i * 128:(i + 1) * 128],
                                W[f"wv{s}"][:, 2 * o:2 * o + 2, :],
                                start=(o == 0), stop=(o == c.DT // 2 - 1),
                                perf_mode=DR)
                        vt = p1s.tile([128, c.NQ], F16, tag="v_out",
                                      name=f"v{s}{i}o")
                        nc.vector.tensor_scalar_mul(
                            vt[:], pv[:], rsqvT[kv][:, i:i + 1])
                        nc.sync.dma_start(
                            sc[f"v16_{s}"][:, ich * (c.TCH // 128) + i, :],
                            vt[:])

        # ===== PHASE 2 + 3 share one pool scope (no barrier between) ======
        with (
            tc.tile_pool(name="p2", bufs=2) as p2,
            tc.tile_pool(name="p2w", bufs=2) as p2w,
            tc.tile_pool(name="p3", bufs=1) as p3,
            tc.tile_pool(name="p3w", bufs=2) as p3w,
            tc.tile_pool(name="p3s", bufs=2) as p3s,
        ):
            def attn_wo(s):
                # attention output lives entirely in SBUF: [feat%128, h, t]
                o8_sb = p2.tile([128, c.NQT, c.T], F8, tag="o8sb", bufs=1,
                                name="o8_sb")
                for b in range(c.B):
                    bsl = slice(b * c.S, (b + 1) * c.S)
                    nblk = c.S // 128
                    for h in range(c.NQT):
                        hsl = slice(h * 128, (h + 1) * 128)
                        kt = p2.tile([128, c.S], F8, tag="kt", bufs=3,
                                     name="kt")
                        nc.sync.dma_start(kt[:], sc[f"k8_{s}"][hsl, bsl])
                        vt = p2.tile([128, nblk, 128], F16, tag="vt", bufs=3,
                                     name="vt")
                        nc.sync.dma_start(
                            vt[:],
                            sc[f"v16_{s}"][:, b * nblk:(b + 1) * nblk, hsl])
                        for q0 in range(0, c.S, c.QCH):
                            qsl = slice(b * c.S + q0, b * c.S + q0 + c.QCH)
                            qt = p2.tile([128, c.QCH], F8, tag="qt", bufs=6,
                                         name="qt")
                            nc.sync.dma_start(qt[:], sc[f"q8_{s}"][hsl, qsl])
                            o_ps = mm([128, c.QCH], "o_ps")
                            sum_ps = row([1, c.QCH], "sum_ps")
                            nk = c.S // 128
                            for ik in range(nk):
                                s_ps = mm([128, c.QCH], "s_ps")
                                nc.tensor.matmul(
                                    s_ps[:], kt[:, ik * 128:(ik + 1) * 128],
                                    qt[:], start=True, stop=True)
                                e16 = p2.tile([128, c.QCH], F16, tag="e16",
                                              bufs=4, name="e16")
                                nc.scalar.activation(e16[:], s_ps[:], AF.Exp,
                                                     scale=EXP_SCALE)
                                nc.tensor.matmul(sum_ps[:], ones_col[:],
                                                 e16[:], start=(ik == 0),
                                                 stop=(ik == nk - 1))
                                nc.tensor.matmul(o_ps[:], vt[:, ik], e16[:],
                                                 start=(ik == 0),
                                                 stop=(ik == nk - 1))
                            rs32 = p2.tile([1, c.QCH], F32, tag="rs32",
                                           name="rs32")
                            nc.vector.reciprocal_approx_fast(rs32[:],
                                                             sum_ps[:])
                            rs16 = p2.tile([1, c.QCH], F16, tag="rs16",
                                           name="rs16")
                            nc.vector.tensor_copy(rs16[:], rs32[:])
                            bc_ps = bc("bc_at")
                            nc.tensor.matmul(bc_ps[:], col_so[:], rs16[:],
                                             start=True, stop=True)
                            bc16 = p2.tile([128, c.QCH], F16, tag="bc16",
                                           name="bc16")
                            nc.vector.tensor_copy(bc16[:], bc_ps[:])
                            nc.vector.tensor_mul(o8_sb[:, h, qsl], o_ps[:],
                                                 bc16[:])

                # ---- wo partial for ALL tokens, window-major ----
                wo_sb = p2w.tile([128, c.NQT, c.D], F8, tag="wo", bufs=1,
                                 name="wo_sb")
                nc.gpsimd.dma_start(wo_sb[:], ins[f"wo8_{s}"][:])
                for w in range(NCORES):
                    for u in range(c.TC // c.TCW):
                        t0 = w * c.TC + u * c.TCW
                        for dt in range(c.DT):
                            hp = mm([128, c.TCW], "hp")
                            nc.tensor.matmul(
                                hp[:],
                                wo_sb[:, :, dt * 128:(dt + 1) * 128],
                                o8_sb[:, :, t0:t0 + c.TCW],
                                start=True, stop=True, perf_mode=DR)
                            hp16 = p2w.tile([128, c.TCW], F16, tag="hp16",
                                            bufs=4, name="hp16")
                            if dt % 2 == 0:
                                nc.vector.tensor_scalar_mul(hp16[:], hp[:],
                                                            hp_col[:])
                            else:
                                nc.scalar.activation(hp16[:], hp[:], AF.Copy,
                                                     scale=HP_SCALE)
                            nc.sync.dma_start(
                                sc[f"hp_{s}"][w * c.D + dt * 128:
                                              w * c.D + (dt + 1) * 128,
                                              u * c.TCW:(u + 1) * c.TCW],
                                hp16[:])

            def rs_trigger(s):
                nc.gpsimd.collective_compute(
                    "ReduceScatter", mybir.AluOpType.add,
                    replica_groups=[list(range(NCORES))],
                    ins=[sc[f"hp_{s}"][:].opt()],
                    outs=[sc[f"h_{s}"][:].opt()],
                )

            P3 = {}

            def p3_prefetch(s, with_h=True, with_w2=True):
                d = P3.setdefault(s, {"w1": {}, "w3": {}, "w2": {}})
                if "fnorm" not in d:
                    fnorm = p3.tile([128, c.DT], F32, tag="fnorm", bufs=2,
                                    name=f"fnorm_{s}")
                    nc.gpsimd.dma_start(fnorm[:], ins[f"fnorm_{s}"])
                    d["fnorm"] = fnorm
                    for fb in range(2):
                        for wn in ("w1", "w3"):
                            t = p3w.tile([128, c.DT, 512], F8, tag=wn,
                                         name=wn)
                            nc.gpsimd.dma_start(
                                t[:], ins[f"{wn}8_{s}"][:, fb, :])
                            d[wn][fb] = t
                if with_w2 and 0 not in d["w2"]:
                    t = p3w.tile([128, c.FT, 256], F8, tag="w2", name="w2")
                    nc.gpsimd.dma_start(t[:], ins[f"w28_{s}"][:, 0, :])
                    d["w2"][0] = t
                if with_h and "h16" not in d:
                    h16 = p3.tile([128, c.DT, c.TCW], F16, tag="h16", bufs=1,
                                  name="h16")
                    nc.gpsimd.dma_start(
                        h16[:],
                        sc[f"h_{s}"][:, :].rearrange("(o p) t -> p o t",
                                                     p=128))
                    d["h16"] = h16

            def ffn_gate(s, fbs):
                p3_prefetch(s)
                d = P3[s]
                tw = c.TCW
                if "h8" not in d:
                    h8 = p3.tile([128, c.DT, tw], F8, tag="h8", name="h8")
                    nc.vector.tensor_copy(h8[:], d["h16"][:])
                    d["h8"] = h8
                    d["zg"] = p3.tile([128, c.FT, tw], F8, tag="zg",
                                      name="zg")
                h8, zg = d["h8"], d["zg"]
                for fb in fbs:
                    tiles = {}
                    for wn in ("w1", "w3"):
                        if fb in d[wn]:
                            tiles[wn] = d[wn].pop(fb)
                        else:
                            t = p3w.tile([128, c.DT, 512], F8, tag=wn,
                                         name=wn)
                            nc.gpsimd.dma_start(
                                t[:], ins[f"{wn}8_{s}"][:, fb, :])
                            tiles[wn] = t
                    for jj in range(4):
                        ft = fb * 4 + jj
                        jsl = slice(jj * 128, (jj + 1) * 128)
                        z1 = mm([128, tw], "z1")
                        z3 = mm([128, tw], "z3")
                        for o in range(c.DT // 2):
                            nc.tensor.matmul(
                                z1[:], tiles["w1"][:, 2 * o:2 * o + 2, jsl],
                                h8[:, 2 * o:2 * o + 2, :],
                                start=(o == 0), stop=(o == c.DT // 2 - 1),
                                perf_mode=DR)
                        for o in range(c.DT // 2):
                            nc.tensor.matmul(
                                z3[:], tiles["w3"][:, 2 * o:2 * o + 2, jsl],
                                h8[:, 2 * o:2 * o + 2, :],
                                start=(o == 0), stop=(o == c.DT // 2 - 1),
                                perf_mode=DR)
                        sg = p3s.tile([128, tw], F16, tag="sg", name="sg")
                        nc.scalar.activation(sg[:], z1[:], AF.Silu,
                                             scale=SILU_SCALE)
                        nc.vector.tensor_mul(zg[:, ft], sg[:], z3[:])

            def ffn_out(s):
                d = P3[s]
                fnorm, zg = d["fnorm"], d["zg"]
                wsl = slice(0, c.TCW)
                tw = c.TCW
                r_all = p3.tile([128, c.DT, tw], F32, tag="r", name="r_all")
                ns_ps = row([1, tw], "ns")
                for db in range(c.DB):
                    if db in d["w2"]:
                        w2 = d["w2"].pop(db)
                    else:
                        w2 = p3w.tile([128, c.FT, 256], F8, tag="w2",
                                      name="w2")
                        nc.gpsimd.dma_start(w2[:], ins[f"w28_{s}"][:, db, :])
                    for sub in range(2):
                        dt = db * 2 + sub
                        jsl = slice(sub * 128, (sub + 1) * 128)
                        fp = mm([128, tw], "fp")
                        for fo in range(c.FT // 2):
                            nc.tensor.matmul(
                                fp[:], w2[:, 2 * fo:2 * fo + 2, jsl],
                                zg[:, 2 * fo:2 * fo + 2, :],
                                start=(fo == 0), stop=(fo == c.FT // 2 - 1),
                                perf_mode=DR)
                        res = p3s.tile([128, tw], F32, tag="res", bufs=2,
                                       name="res")
                        nc.gpsimd.dma_start(
                            res[:],
                            ins[f"res_{s}"][dt * 128:(dt + 1) * 128, wsl])
                        nc.vector.tensor_add(r_all[:, dt], fp[:], res[:])
                        r2 = p3s.tile([128, tw], BF16, tag="r2", name="r2")
                        eng = nc.vector if dt % 2 == 0 else nc.gpsimd
                        eng.tensor_mul(r2[:], r_all[:, dt], r_all[:, dt])
                        nc.tensor.matmul(ns_ps[:], onesb_col[:], r2[:],
                                         start=(dt == 0),
                                         stop=(dt == c.DT - 1))
                rmsn = p3s.tile([1, tw], F32, tag="rmsn", name="rmsn")
                nc.scalar.activation(rmsn[:], ns_ps[:], AF.Sqrt,
                                     bias=eps1[:],
                                     scale=1.0 / (c.D * S_FF * S_FF))
                rsqn = p3s.tile([1, tw], F32, tag="rsqn", name="rsqn")
                nc.vector.reciprocal_approx_fast(rsqn[:], rmsn[:])
                bcn_ps = bc("bcn")
                nc.tensor.matmul(bcn_ps[:], col_fn[:], rsqn[:],
                                 start=True, stop=True)
                for dt in range(c.DT):
                    rn = p3s.tile([128, tw], F32, tag="rn", name="rn")
                    nc.vector.tensor_mul(rn[:], r_all[:, dt], bcn_ps[:])
                    ofn = p3s.tile([128, tw], F32, tag="ofn", name="ofn")
                    nc.scalar.activation(ofn[:], rn[:], AF.Copy,
                                         scale=fnorm[:, dt:dt + 1])
                    nc.sync.dma_start(
                        outs[s][dt * 128:(dt + 1) * 128, wsl], ofn[:])

            attn_wo("x")
            rs_trigger("x")
            attn_wo("y")
            p3_prefetch("x")     # x-stream FFN loads issued before RS_y
            ffn_gate("x", range(0, 3))
            rs_trigger("y")      # RS_y overlaps the rest of FFN-x
            ffn_gate("x", range(3, c.FB))
            p3_prefetch("y", with_h=False, with_w2=False)
            ffn_out("x")
            p3_prefetch("y")
            ffn_gate("y", range(c.FB))
            ffn_out("y")


# ======================= host-side wrapper =========================

_CACHE = {}


def _to_f8(a, scale):
    import ml_dtypes
    a = np.asarray(a, np.float32) * scale
    np.clip(a, -240.0, 240.0, out=a)
    return a.astype(ml_dtypes.float8_e4m3)


def _prep_inputs(cfg, x, y, attn_norm_w,
                 wq_x, wk_x, wv_x, wo_x, wq_y, wk_y, wv_y, wo_y,
                 w1_x, w2_x, w3_x, ffn_norm_x,
                 w1_y, w2_y, w3_y, ffn_norm_y):
    c = cfg
    nw = np.asarray(attn_norm_w, np.float32)
    qscale = nw / np.sqrt(c.HD)

    per_core = [dict() for _ in range(NCORES)]
    shared = {}
    for s, (xv, wq, wk, wv, wo, w1, w2, w3, fn) in {
        "x": (x, wq_x, wk_x, wv_x, wo_x, w1_x, w2_x, w3_x, ffn_norm_x),
        "y": (y, wq_y, wk_y, wv_y, wo_y, w1_y, w2_y, w3_y, ffn_norm_y),
    }.items():
        xf = np.asarray(xv, np.float32).reshape(c.T, c.D)
        # [p, o, t] swizzle
        shared[f"{s}T8"] = _to_f8(
            np.ascontiguousarray(xf.reshape(c.T, c.DT, 128)
                                 .transpose(2, 1, 0)), S_X)
        wqT = (np.asarray(wq, np.float32) * qscale[None, :]).T  # [D, D]
        wkT = (np.asarray(wk, np.float32) * nw[None, :]).T
        wvT = (np.asarray(wv, np.float32) * nw[None, :]).T
        woT = np.asarray(wo, np.float32).T                      # [Din, Dout]
        w1T = np.asarray(w1, np.float32).T                      # [D, FF]
        w3T = np.asarray(w3, np.float32).T
        w2T = np.asarray(w2, np.float32).T                      # [FF, D]
        # w1/w3: [p, fb, o, j512] ; w2: [p, db, fo, j256]
        shared[f"w18_{s}"] = _to_f8(
            w1T.reshape(c.DT, 128, c.FB, 512).transpose(1, 2, 0, 3)
            .reshape(128, c.FB, c.DT * 512), S_W1)
        shared[f"w38_{s}"] = _to_f8(
            w3T.reshape(c.DT, 128, c.FB, 512).transpose(1, 2, 0, 3)
            .reshape(128, c.FB, c.DT * 512), S_W3)
        shared[f"w28_{s}"] = _to_f8(
            w2T.reshape(c.FT, 128, c.DB, 256).transpose(1, 2, 0, 3)
            .reshape(128, c.DB, c.FT * 256), S_W2)
        shared[f"fnorm_{s}"] = np.ascontiguousarray(
            np.asarray(fn, np.float32).reshape(c.DT, 128).T)
        xt = xf.T  # [D, T]
        for r in range(NCORES):
            js = slice(r * c.NQ, (r + 1) * c.NQ)
            ts = slice(r * c.TC, (r + 1) * c.TC)
            per_core[r][f"wq8_{s}"] = _to_f8(
                wqT[:, js].reshape(c.DT, 128, c.NQ).transpose(1, 0, 2), S_WQ)
            per_core[r][f"wk8_{s}"] = _to_f8(
                wkT[:, js].reshape(c.DT, 128, c.NQ).transpose(1, 0, 2), S_WK)
            per_core[r][f"wv8_{s}"] = _to_f8(
                wvT[:, js].reshape(c.DT, 128, c.NQ).transpose(1, 0, 2), S_WV)
            per_core[r][f"wo8_{s}"] = _to_f8(
                woT[js, :].reshape(c.NQT, 128, c.D).transpose(1, 0, 2), S_WO)
            per_core[r][f"res_{s}"] = np.ascontiguousarray(
                xt[:, ts]) * np.float32(S_FF)
    in_maps = []
    for r in range(NCORES):
        m = dict(shared)
        m.update(per_core[r])
        in_maps.append(m)
    return in_maps


def run(cfg, inputs, **kw):
    from concourse import bass_utils

    key = (cfg.B, cfg.S, cfg.D, cfg.H, cfg.HD, cfg.FF)
    if key not in _CACHE:
        _CACHE[key] = build(cfg)
    nc = _CACHE[key]
    in_maps = _prep_inputs(cfg, **{k: v for k, v in inputs.items()
                                   if k != "start_pos"})
    res = bass_utils.run_bass_kernel_spmd(
        nc, in_maps, core_ids=list(range(NCORES)), **kw)
    outs = []
    for s in ("x", "y"):
        cols = [res.results[r][f"out_{s}"] for r in range(NCORES)]
        full_t = np.concatenate(cols, axis=1)           # [D, T]
        outs.append(np.ascontiguousarray(full_t.T)
                    .reshape(cfg.B, cfg.S, cfg.D).astype(np.float32))
    return tuple(outs), res


def kernel(**inputs):
    (out_x, out_y), _ = run(FULL, inputs)
    return out_x, out_y


# revision 30
# speedup vs baseline: 1.0520x; 1.0520x over previous
"""CrossAttentionBlockLLaMA on 8 Trainium2 NeuronCores (Bass/Tile).

Sharding (unchanged from baseline):
  - QKV + attention: tensor-parallel over heads (2 heads/core).
  - wo: row-sharded; per-core partial h for ALL tokens written window-major
    [8, D, TC]; ReduceScatter sums partials -> core r owns tokens_r.
  - FFN + post-norm: token-parallel (TC tokens/core), full weights.

Perf changes vs baseline:
  - fp8 e4m3 DoubleRow matmuls for QKV / wo / FFN (1.44x PE), scores in
    fp8 normal mode, PV/sum in fp16. Power-of-2 static scales keep all
    tensors centered in e4m3 range; final output path stays fp32.
    (out ~= x + ff with ||ff|| ~ 4e-4*||x||, so the fp8 noise on the
    attention+FFN path is attenuated ~2000x in the output.)
  - All row->128-partition broadcasts done with K=1 PE matmuls instead of
    DRAM bounce DMAs (the bounces deadlocked behind the ReduceScatter).
  - rsqrt/silu/reciprocal folded into single scalar ACTs.
  - Host pre-swizzles every weight/activation into its SBUF layout so all
    DMA loads are contiguous; weight streams ride the gpsimd queue.
  - Phase-3(x) inputs prefetched before ReduceScatter(y) is triggered.

Self-contained: hardcodes shapes from the problem spec.
"""
import numpy as np

NCORES = 8
EPS = 1e-5

# power-of-2 static scales (inputs are randn; generous sigma margins)
S_X = 2.0 ** 4     # x,y activations
S_WQ = 2.0 ** 14   # folded wq (incl attn_norm_w and 1/sqrt(HD))
S_WK = 2.0 ** 10
S_WV = 2.0 ** 10
S_Q = 2.0 ** 8     # q8 scratch
S_K = 2.0 ** 5     # k8 scratch
S_V = 2.0 ** 6     # v16 scratch carries rsq*2^6
S_O = 2.0 ** 7     # o8 (normalized attention out)
S_WO = 2.0 ** 10
S_H = 2.0 ** 4     # h after ReduceScatter (folded into hp copy)
S_W1 = 2.0 ** 9
S_W3 = 2.0 ** 8
S_W2 = 2.0 ** 9
S_ZG = S_W3 * S_H          # 2^13, implicit scale of zg8
S_FF = S_W2 * S_ZG         # 2^22, scale of ffn psum and residual

C_Q = S_Q / (S_WQ * S_X)   # 2^-10  (replicate col for q)
C_K = S_K / (S_WK * S_X)   # 2^-9
C_V = S_V / (S_WV * S_X)   # 2^-8   (folded into rsqv row)
C_SO = S_O / S_V           # 2^3    (replicate col for o normalize)
EXP_SCALE = 1.0 / (S_Q * S_K)      # 2^-13
HP_SCALE = S_H / (S_WO * S_O)      # 2^-15
SILU_SCALE = 1.0 / (S_W1 * S_H)    # 2^-13


class Cfg:
    def __init__(self, B=2, S=2048, D=2048, H=16, HD=128, FF=5632):
        self.B, self.S, self.D, self.H, self.HD, self.FF = B, S, D, H, HD, FF
        self.T = B * S                    # total tokens
        self.TC = self.T // NCORES        # tokens per core (phase 3)
        self.NQ = (H // NCORES) * HD      # per-core head dims
        self.DT = D // 128                # d-tiles
        self.FT = FF // 128               # ff-tiles
        self.NQT = self.NQ // 128         # per-core head-dim tiles
        self.TCH = min(512, self.T)       # phase-1 token chunk
        self.QCH = min(512, S)            # phase-2 query chunk
        self.TCW = min(512, self.TC)      # phase-3 / wo token chunk
        self.FB = self.FT // 4            # w1/w3 column groups (512 wide)
        self.DB = self.DT // 2            # w2 column groups (256 wide)
        assert self.T % self.TCH == 0 and S % self.QCH == 0
        assert self.TC % self.TCW == 0 and S % 128 == 0
        assert HD == 128 and D % 128 == 0 and FF % 128 == 0
        assert self.DT % 2 == 0 and self.FT % 4 == 0 and self.NQT == 2


FULL = Cfg()


def build(cfg=FULL):
    import concourse.mybir as mybir
    import concourse.tile as tile
    from concourse import bacc

    F8 = mybir.dt.float8e4
    F16 = mybir.dt.float16
    F32 = mybir.dt.float32

    c = cfg
    nc = bacc.Bacc("TRN2", target_bir_lowering=False, debug=False,
                   num_devices=NCORES)

    ins = {}
    outs = {}
    for s in ("x", "y"):
        # activations pre-swizzled [p, o, t], *S_X, fp8
        ins[f"{s}T8"] = nc.dram_tensor(f"{s}T8", [128, c.DT, c.T], F8,
                                       kind="ExternalInput").ap()
        for w in ("wq", "wk", "wv"):
            ins[f"{w}8_{s}"] = nc.dram_tensor(
                f"{w}8_{s}", [128, c.DT, c.NQ], F8, kind="ExternalInput").ap()
        ins[f"wo8_{s}"] = nc.dram_tensor(
            f"wo8_{s}", [128, c.NQT, c.D], F8, kind="ExternalInput").ap()
        ins[f"w18_{s}"] = nc.dram_tensor(
            f"w18_{s}", [128, c.FB, c.DT * 512], F8, kind="ExternalInput").ap()
        ins[f"w38_{s}"] = nc.dram_tensor(
            f"w38_{s}", [128, c.FB, c.DT * 512], F8, kind="ExternalInput").ap()
        ins[f"w28_{s}"] = nc.dram_tensor(
            f"w28_{s}", [128, c.DB, c.FT * 256], F8, kind="ExternalInput").ap()
        ins[f"res_{s}"] = nc.dram_tensor(
            f"res_{s}", [c.D, c.TC], F32, kind="ExternalInput").ap()
        ins[f"fnorm_{s}"] = nc.dram_tensor(
            f"fnorm_{s}", [128, c.DT], F32, kind="ExternalInput").ap()
        outs[s] = nc.dram_tensor(f"out_{s}", [c.D, c.TC], F32,
                                 kind="ExternalOutput").ap()

    with tile.TileContext(nc) as tc:
        _emit(tc, nc, c, ins, outs)
    nc.compile()
    return nc


def _emit(tc, nc, c, ins, outs):
    import concourse.mybir as mybir

    F8 = mybir.dt.float8e4
    F16 = mybir.dt.float16
    BF16 = mybir.dt.bfloat16
    F32 = mybir.dt.float32
    AF = mybir.ActivationFunctionType
    DR = mybir.MatmulPerfMode.DoubleRow

    with (
        tc.tile_pool(name="psum", bufs=1, space="PSUM") as ps,
        tc.tile_pool(name="const", bufs=1) as const,
        tc.tile_pool(name="dram", bufs=1, space="DRAM") as dram,
    ):
        ones_col = const.tile([128, 1], F16)
        nc.vector.memset(ones_col[:], 1.0)
        onesb_col = const.tile([128, 1], BF16)
        nc.vector.memset(onesb_col[:], 1.0)
        col_cq = const.tile([1, 128], F16)
        nc.vector.memset(col_cq[:], C_Q)
        col_ck = const.tile([1, 128], F16)
        nc.vector.memset(col_ck[:], C_K)
        col_so = const.tile([1, 128], F16)
        nc.vector.memset(col_so[:], C_SO)
        col_fn = const.tile([1, 128], F32)
        nc.vector.memset(col_fn[:], 1.0 / S_FF)
        hp_col = const.tile([128, 1], F32)
        nc.vector.memset(hp_col[:], HP_SCALE)
        eps1 = const.tile([1, 1], F32)
        nc.vector.memset(eps1[:], EPS)

        sc = {}
        for s in ("x", "y"):
            sc[f"q8_{s}"] = dram.tile([c.NQ, c.T], F8, name=f"q8_{s}")
            sc[f"k8_{s}"] = dram.tile([c.NQ, c.T], F8, name=f"k8_{s}")
            # v16: [token%128, token-block, per-core feature]
            sc[f"v16_{s}"] = dram.tile([128, c.T // 128, c.NQ], F16,
                                       name=f"v16_{s}")

            sc[f"hp_{s}"] = dram.tile([NCORES * c.D, c.TC], F16,
                                      name=f"hp_{s}")
            sc[f"h_{s}"] = dram.tile([c.D, c.TC], F16, name=f"h_{s}")
            sc[f"rvd_{s}"] = dram.tile([1, c.TCH], F32, tag=f"rvd_{s}",
                                       bufs=2, name=f"rvd_{s}")

        def mm(shape, name):
            return ps.tile(shape, F32, tag="mm", bufs=4, name=name)

        def row(shape, name):
            return ps.tile(shape, F32, tag="row", bufs=2, name=name)

        def bc(name, dtype=F32):
            return ps.tile([128, 512], dtype, tag="bc", bufs=2, name=name)

        # ============ PHASE 1: RMSNorm stats + QKV projections =============
        with (
            tc.tile_pool(name="p1w", bufs=1) as p1w,
            tc.tile_pool(name="p1a", bufs=2) as p1a,
            tc.tile_pool(name="p1s", bufs=3) as p1s,
        ):
            W = {}
            for s in ("x", "y"):
                for w in ("wq", "wk", "wv"):
                    t = p1w.tile([128, c.DT, c.NQ], F8, name=f"{w}_{s}_sb")
                    nc.gpsimd.dma_start(t[:], ins[f"{w}8_{s}"][:])
                    W[f"{w}{s}"] = t

            for ich in range(c.T // c.TCH):
                tsl = slice(ich * c.TCH, (ich + 1) * c.TCH)
                act = {}
                rsq16 = {}
                rsqvT = {}
                for s in ("x", "y"):
                    at = p1a.tile([128, c.DT, c.TCH], F8, tag=f"act_{s}",
                                  name=f"act_{s}")
                    nc.sync.dma_start(at[:], ins[f"{s}T8"][:, :, tsl])
                    act[s] = at

                    ms_ps = row([1, c.TCH], f"ms_{s}")
                    for o in range(c.DT):
                        sq = p1s.tile([128, c.TCH], F16, tag="sq",
                                      name=f"sq_{s}{o}")
                        eng = nc.vector if o % 2 == 0 else nc.gpsimd
                        eng.tensor_mul(sq[:], at[:, o], at[:, o])
                        nc.tensor.matmul(ms_ps[:], ones_col[:], sq[:],
                                         start=(o == 0), stop=(o == c.DT - 1))
                    # rms -> 1/rms (fast approx) ; rsqv = C_V * rsq
                    rms32 = p1s.tile([1, c.TCH], F32, tag="rms",
                                     name=f"rms_{s}")
                    nc.scalar.activation(rms32[:], ms_ps[:], AF.Sqrt,
                                         bias=eps1[:],
                                         scale=1.0 / (c.D * S_X * S_X))
                    rsq32 = p1s.tile([1, c.TCH], F32, tag="rsq32",
                                     name=f"rq32_{s}")
                    nc.vector.reciprocal_approx_fast(rsq32[:], rms32[:])
                    r16 = p1s.tile([1, c.TCH], F16, tag="rsq", name=f"rq_{s}")
                    nc.vector.tensor_copy(r16[:], rsq32[:])
                    rsq16[s] = r16
                    rv32 = p1s.tile([1, c.TCH], F32, tag="rsqv",
                                    name=f"rv_{s}")
                    nc.scalar.activation(rv32[:], rsq32[:], AF.Copy,
                                         scale=C_V)
                    # bounce rsqv row -> [128, nsub] per-partition scalars
                    nc.sync.dma_start(sc[f"rvd_{s}"][:], rv32[:])
                    rT = p1s.tile([128, c.TCH // 128], F32, tag="rsqT",
                                  name=f"rT_{s}")
                    nc.sync.dma_start(
                        rT[:],
                        sc[f"rvd_{s}"][0, :].rearrange("(n p) -> p n", p=128))
                    rsqvT[s] = rT

                for s in ("x", "y"):
                    kv = "y" if s == "x" else "x"
                    bq_ps = bc(f"bq_{s}")
                    nc.tensor.matmul(bq_ps[:], col_cq[:], rsq16[s][:],
                                     start=True, stop=True)
                    bq16 = p1s.tile([128, c.TCH], F16, tag="bq16",
                                    name=f"bq16_{s}")
                    nc.vector.tensor_copy(bq16[:], bq_ps[:])
                    bk_ps = bc(f"bk_{s}")
                    nc.tensor.matmul(bk_ps[:], col_ck[:], rsq16[kv][:],
                                     start=True, stop=True)
                    bk16 = p1s.tile([128, c.TCH], F16, tag="bk16",
                                    name=f"bk16_{s}")
                    nc.vector.tensor_copy(bk16[:], bk_ps[:])

                    for (wname, src, bct, dst) in (
                        ("wq", s, bq16, sc[f"q8_{s}"]),
                        ("wk", kv, bk16, sc[f"k8_{s}"]),
                    ):
                        wt = W[f"{wname}{s}"]
                        for jt in range(c.NQT):
                            pm = mm([128, c.TCH], f"{wname}{s}{jt}")
                            for o in range(c.DT // 2):
                                nc.tensor.matmul(
                                    pm[:],
                                    wt[:, 2 * o:2 * o + 2,
                                       jt * 128:(jt + 1) * 128],
                                    act[src][:, 2 * o:2 * o + 2, :],
                                    start=(o == 0), stop=(o == c.DT // 2 - 1),
                                    perf_mode=DR)
                            ot8 = p1s.tile([128, c.TCH], F8, tag="proj_out",
                                           name=f"{wname}{s}{jt}o")
                            nc.vector.tensor_mul(ot8[:], pm[:], bct[:])
                            nc.sync.dma_start(
                                dst[jt * 128:(jt + 1) * 128, tsl], ot8[:])

                    for i in range(c.TCH // 128):
                        pv = mm([128, c.NQ], f"v{s}{i}")
                        for o in range(c.DT // 2):
                            nc.tensor.matmul(
                                pv[:],
                                act[kv][:, 2 * o:2 * o + 2,
                                        i * 128:(i + 1) * 128],
                                W[f"wv{s}"][:, 2 * o:2 * o + 2, :],
                                start=(o == 0), stop=(o == c.DT // 2 - 1),
                                perf_mode=DR)
                        vt = p1s.tile([128, c.NQ], F16, tag="v_out",
                                      name=f"v{s}{i}o")
                        nc.vector.tensor_scalar_mul(
                            vt[:], pv[:], rsqvT[kv][:, i:i + 1])
                        nc.sync.dma_start(
                            sc[f"v16_{s}"][:, ich * (c.TCH // 128) + i, :],
                            vt[:])

        # ===== PHASE 2 + 3 share one pool scope (no barrier between) ======
        with (
            tc.tile_pool(name="p2", bufs=2) as p2,
            tc.tile_pool(name="p2w", bufs=2) as p2w,
            tc.tile_pool(name="p3", bufs=1) as p3,
            tc.tile_pool(name="p3w", bufs=2) as p3w,
            tc.tile_pool(name="p3s", bufs=2) as p3s,
        ):
            def attn_wo(s):
                # attention output lives entirely in SBUF: [feat%128, h, t]
                o8_sb = p2.tile([128, c.NQT, c.T], F8, tag="o8sb", bufs=1,
                                name="o8_sb")
                for b in range(c.B):
                    bsl = slice(b * c.S, (b + 1) * c.S)
                    nblk = c.S // 128
                    for h in range(c.NQT):
                        hsl = slice(h * 128, (h + 1) * 128)
                        kt = p2.tile([128, c.S], F8, tag="kt", bufs=3,
                                     name="kt")
                        nc.sync.dma_start(kt[:], sc[f"k8_{s}"][hsl, bsl])
                        vt = p2.tile([128, nblk, 128], F16, tag="vt", bufs=3,
                                     name="vt")
                        nc.sync.dma_start(
                            vt[:],
                            sc[f"v16_{s}"][:, b * nblk:(b + 1) * nblk, hsl])
                        for q0 in range(0, c.S, c.QCH):
                            qsl = slice(b * c.S + q0, b * c.S + q0 + c.QCH)
                            qt = p2.tile([128, c.QCH], F8, tag="qt", bufs=6,
                                         name="qt")
                            nc.sync.dma_start(qt[:], sc[f"q8_{s}"][hsl, qsl])
                            o_ps = mm([128, c.QCH], "o_ps")
                            sum_ps = row([1, c.QCH], "sum_ps")
                            nk = c.S // 128
                            for ik in range(nk):
                                s_ps = mm([128, c.QCH], "s_ps")
                                nc.tensor.matmul(
                                    s_ps[:], kt[:, ik * 128:(ik + 1) * 128],
                                    qt[:], start=True, stop=True)
                                e16 = p2.tile([128, c.QCH], F16, tag="e16",
                                              bufs=4, name="e16")
                                nc.scalar.activation(e16[:], s_ps[:], AF.Exp,
                                                     scale=EXP_SCALE)
                                nc.tensor.matmul(sum_ps[:], ones_col[:],
                                                 e16[:], start=(ik == 0),
                                                 stop=(ik == nk - 1))
                                nc.tensor.matmul(o_ps[:], vt[:, ik], e16[:],
                                                 start=(ik == 0),
                                                 stop=(ik == nk - 1))
                            rs32 = p2.tile([1, c.QCH], F32, tag="rs32",
                                           name="rs32")
                            nc.vector.reciprocal_approx_fast(rs32[:],
                                                             sum_ps[:])
                            rs16 = p2.tile([1, c.QCH], F16, tag="rs16",
                                           name="rs16")
                            nc.vector.tensor_copy(rs16[:], rs32[:])
                            bc_ps = bc("bc_at")
                            nc.tensor.matmul(bc_ps[:], col_so[:], rs16[:],
                                             start=True, stop=True)
                            bc16 = p2.tile([128, c.QCH], F16, tag="bc16",
                                           name="bc16")
                            nc.vector.tensor_copy(bc16[:], bc_ps[:])
                            nc.vector.tensor_mul(o8_sb[:, h, qsl], o_ps[:],
                                                 bc16[:])

                # ---- wo partial for ALL tokens, window-major ----
                wo_sb = p2w.tile([128, c.NQT, c.D], F8, tag="wo", bufs=1,
                                 name="wo_sb")
                nc.gpsimd.dma_start(wo_sb[:], ins[f"wo8_{s}"][:])
                for w in range(NCORES):
                    for u in range(c.TC // c.TCW):
                        t0 = w * c.TC + u * c.TCW
                        for dt in range(c.DT):
                            hp = mm([128, c.TCW], "hp")
                            nc.tensor.matmul(
                                hp[:],
                                wo_sb[:, :, dt * 128:(dt + 1) * 128],
                                o8_sb[:, :, t0:t0 + c.TCW],
                                start=True, stop=True, perf_mode=DR)
                            hp16 = p2w.tile([128, c.TCW], F16, tag="hp16",
                                            bufs=4, name="hp16")
                            if dt % 2 == 0:
                                nc.vector.tensor_scalar_mul(hp16[:], hp[:],
                                                            hp_col[:])
                            else:
                                nc.scalar.activation(hp16[:], hp[:], AF.Copy,
                                                     scale=HP_SCALE)
                            nc.sync.dma_start(
                                sc[f"hp_{s}"][w * c.D + dt * 128:
                                              w * c.D + (dt + 1) * 128,
                                              u * c.TCW:(u + 1) * c.TCW],
                                hp16[:])

            def rs_trigger(s):
                nc.gpsimd.collective_compute(
                    "ReduceScatter", mybir.AluOpType.add,
                    replica_groups=[list(range(NCORES))],
                    ins=[sc[f"hp_{s}"][:].opt()],
                    outs=[sc[f"h_{s}"][:].opt()],
                )

            P3 = {}

            def p3_prefetch(s, with_h=True, with_w2=True):
                d = P3.setdefault(s, {"w1": {}, "w3": {}, "w2": {}})
                if "fnorm" not in d:
                    fnorm = p3.tile([128, c.DT], F32, tag="fnorm", bufs=2,
                                    name=f"fnorm_{s}")
                    nc.gpsimd.dma_start(fnorm[:], ins[f"fnorm_{s}"])
                    d["fnorm"] = fnorm
                    for fb in range(2):
                        for wn in ("w1", "w3"):
                            t = p3w.tile([128, c.DT, 512], F8, tag=wn,
                                         name=wn)
                            nc.gpsimd.dma_start(
                                t[:], ins[f"{wn}8_{s}"][:, fb, :])
                            d[wn][fb] = t
                if with_w2 and 0 not in d["w2"]:
                    t = p3w.tile([128, c.FT, 256], F8, tag="w2", name="w2")
                    nc.gpsimd.dma_start(t[:], ins[f"w28_{s}"][:, 0, :])
                    d["w2"][0] = t
                if with_h and "h16" not in d:
                    h16 = p3.tile([128, c.DT, c.TCW], F16, tag="h16", bufs=1,
                                  name="h16")
                    nc.gpsimd.dma_start(
                        h16[:],
                        sc[f"h_{s}"][:, :].rearrange("(o p) t -> p o t",
                                                     p=128))
                    d["h16"] = h16

            def ffn_gate(s, fbs):
                p3_prefetch(s)
                d = P3[s]
                tw = c.TCW
                if "h8" not in d:
                    h8 = p3.tile([128, c.DT, tw], F8, tag="h8", name="h8")
                    # gpsimd, NOT vector: this cast waits on the
                    # ReduceScatter (via h16); on the strict-FIFO vector
                    # queue it would block attention/FFN vector ops that
                    # were scheduled behind it.
                    nc.gpsimd.tensor_copy(h8[:], d["h16"][:])
                    d["h8"] = h8
                    d["zg"] = p3.tile([128, c.FT, tw], F8, tag="zg",
                                      name="zg")
                h8, zg = d["h8"], d["zg"]
                for fb in fbs:
                    tiles = {}
                    for wn in ("w1", "w3"):
                        if fb in d[wn]:
                            tiles[wn] = d[wn].pop(fb)
                        else:
                            t = p3w.tile([128, c.DT, 512], F8, tag=wn,
                                         name=wn)
                            nc.gpsimd.dma_start(
                                t[:], ins[f"{wn}8_{s}"][:, fb, :])
                            tiles[wn] = t
                    for jj in range(4):
                        ft = fb * 4 + jj
                        jsl = slice(jj * 128, (jj + 1) * 128)
                        z1 = mm([128, tw], "z1")
                        z3 = mm([128, tw], "z3")
                        for o in range(c.DT // 2):
                            nc.tensor.matmul(
                                z1[:], tiles["w1"][:, 2 * o:2 * o + 2, jsl],
                                h8[:, 2 * o:2 * o + 2, :],
                                start=(o == 0), stop=(o == c.DT // 2 - 1),
                                perf_mode=DR)
                        for o in range(c.DT // 2):
                            nc.tensor.matmul(
                                z3[:], tiles["w3"][:, 2 * o:2 * o + 2, jsl],
                                h8[:, 2 * o:2 * o + 2, :],
                                start=(o == 0), stop=(o == c.DT // 2 - 1),
                                perf_mode=DR)
                        sg = p3s.tile([128, tw], F16, tag="sg", name="sg")
                        nc.scalar.activation(sg[:], z1[:], AF.Silu,
                                             scale=SILU_SCALE)
                        nc.vector.tensor_mul(zg[:, ft], sg[:], z3[:])

            def ffn_out(s):
                d = P3[s]
                fnorm, zg = d["fnorm"], d["zg"]
                wsl = slice(0, c.TCW)
                tw = c.TCW
                r_all = p3.tile([128, c.DT, tw], F32, tag="r", name="r_all")
                ns_ps = row([1, tw], "ns")
                for db in range(c.DB):
                    if db in d["w2"]:
                        w2 = d["w2"].pop(db)
                    else:
                        w2 = p3w.tile([128, c.FT, 256], F8, tag="w2",
                                      name="w2")
                        nc.gpsimd.dma_start(w2[:], ins[f"w28_{s}"][:, db, :])
                    for sub in range(2):
                        dt = db * 2 + sub
                        jsl = slice(sub * 128, (sub + 1) * 128)
                        fp = mm([128, tw], "fp")
                        for fo in range(c.FT // 2):
                            nc.tensor.matmul(
                                fp[:], w2[:, 2 * fo:2 * fo + 2, jsl],
                                zg[:, 2 * fo:2 * fo + 2, :],
                                start=(fo == 0), stop=(fo == c.FT // 2 - 1),
                                perf_mode=DR)
                        res = p3s.tile([128, tw], F32, tag="res", bufs=2,
                                       name="res")
                        nc.gpsimd.dma_start(
                            res[:],
                            ins[f"res_{s}"][dt * 128:(dt + 1) * 128, wsl])
                        nc.vector.tensor_add(r_all[:, dt], fp[:], res[:])
                        r2 = p3s.tile([128, tw], BF16, tag="r2", name="r2")
                        eng = nc.vector if dt % 2 == 0 else nc.gpsimd
                        eng.tensor_mul(r2[:], r_all[:, dt], r_all[:, dt])
                        nc.tensor.matmul(ns_ps[:], onesb_col[:], r2[:],
                                         start=(dt == 0),
                                         stop=(dt == c.DT - 1))
                rmsn = p3s.tile([1, tw], F32, tag="rmsn", name="rmsn")
                nc.scalar.activation(rmsn[:], ns_ps[:], AF.Sqrt,
                                     bias=eps1[:],
                                     scale=1.0 / (c.D * S_FF * S_FF))
                rsqn = p3s.tile([1, tw], F32, tag="rsqn", name="rsqn")
                nc.vector.reciprocal_approx_fast(rsqn[:], rmsn[:])
                bcn_ps = bc("bcn")
                nc.tensor.matmul(bcn_ps[:], col_fn[:], rsqn[:],
                                 start=True, stop=True)
                for dt in range(c.DT):
                    rn = p3s.tile([128, tw], F32, tag="rn", name="rn")
                    nc.vector.tensor_mul(rn[:], r_all[:, dt], bcn_ps[:])
                    ofn = p3s.tile([128, tw], F32, tag="ofn", name="ofn")
                    nc.scalar.activation(ofn[:], rn[:], AF.Copy,
                                         scale=fnorm[:, dt:dt + 1])
                    nc.sync.dma_start(
                        outs[s][dt * 128:(dt + 1) * 128, wsl], ofn[:])

            attn_wo("x")
            rs_trigger("x")
            attn_wo("y")
            p3_prefetch("x")     # x-stream FFN loads issued before RS_y
            ffn_gate("x", range(0, 3))
            rs_trigger("y")      # RS_y overlaps the rest of FFN-x
            ffn_gate("x", range(3, c.FB))
            p3_prefetch("y", with_h=False, with_w2=False)
            ffn_out("x")
            p3_prefetch("y")
            ffn_gate("y", range(c.FB))
            ffn_out("y")


# ======================= host-side wrapper =========================

_CACHE = {}


def _to_f8(a, scale):
    import ml_dtypes
    a = np.asarray(a, np.float32) * scale
    np.clip(a, -240.0, 240.0, out=a)
    return a.astype(ml_dtypes.float8_e4m3)


def _prep_inputs(cfg, x, y, attn_norm_w,
                 wq_x, wk_x, wv_x, wo_x, wq_y, wk_y, wv_y, wo_y,
                 w1_x, w2_x, w3_x, ffn_norm_x,
                 w1_y, w2_y, w3_y, ffn_norm_y):
    c = cfg
    nw = np.asarray(attn_norm_w, np.float32)
    qscale = nw / np.sqrt(c.HD)

    per_core = [dict() for _ in range(NCORES)]
    shared = {}
    for s, (xv, wq, wk, wv, wo, w1, w2, w3, fn) in {
        "x": (x, wq_x, wk_x, wv_x, wo_x, w1_x, w2_x, w3_x, ffn_norm_x),
        "y": (y, wq_y, wk_y, wv_y, wo_y, w1_y, w2_y, w3_y, ffn_norm_y),
    }.items():
        xf = np.asarray(xv, np.float32).reshape(c.T, c.D)
        # [p, o, t] swizzle
        shared[f"{s}T8"] = _to_f8(
            np.ascontiguousarray(xf.reshape(c.T, c.DT, 128)
                                 .transpose(2, 1, 0)), S_X)
        wqT = (np.asarray(wq, np.float32) * qscale[None, :]).T  # [D, D]
        wkT = (np.asarray(wk, np.float32) * nw[None, :]).T
        wvT = (np.asarray(wv, np.float32) * nw[None, :]).T
        woT = np.asarray(wo, np.float32).T                      # [Din, Dout]
        w1T = np.asarray(w1, np.float32).T                      # [D, FF]
        w3T = np.asarray(w3, np.float32).T
        w2T = np.asarray(w2, np.float32).T                      # [FF, D]
        # w1/w3: [p, fb, o, j512] ; w2: [p, db, fo, j256]
        shared[f"w18_{s}"] = _to_f8(
            w1T.reshape(c.DT, 128, c.FB, 512).transpose(1, 2, 0, 3)
            .reshape(128, c.FB, c.DT * 512), S_W1)
        shared[f"w38_{s}"] = _to_f8(
            w3T.reshape(c.DT, 128, c.FB, 512).transpose(1, 2, 0, 3)
            .reshape(128, c.FB, c.DT * 512), S_W3)
        shared[f"w28_{s}"] = _to_f8(
            w2T.reshape(c.FT, 128, c.DB, 256).transpose(1, 2, 0, 3)
            .reshape(128, c.DB, c.FT * 256), S_W2)
        shared[f"fnorm_{s}"] = np.ascontiguousarray(
            np.asarray(fn, np.float32).reshape(c.DT, 128).T)
        xt = xf.T  # [D, T]
        for r in range(NCORES):
            js = slice(r * c.NQ, (r + 1) * c.NQ)
            ts = slice(r * c.TC, (r + 1) * c.TC)
            per_core[r][f"wq8_{s}"] = _to_f8(
                wqT[:, js].reshape(c.DT, 128, c.NQ).transpose(1, 0, 2), S_WQ)
            per_core[r][f"wk8_{s}"] = _to_f8(
                wkT[:, js].reshape(c.DT, 128, c.NQ).transpose(1, 0, 2), S_WK)
            per_core[r][f"wv8_{s}"] = _to_f8(
                wvT[:, js].reshape(c.DT, 128, c.NQ).transpose(1, 0, 2), S_WV)
            per_core[r][f"wo8_{s}"] = _to_f8(
                woT[js, :].reshape(c.NQT, 128, c.D).transpose(1, 0, 2), S_WO)
            per_core[r][f"res_{s}"] = np.ascontiguousarray(
                xt[:, ts]) * np.float32(S_FF)
    in_maps = []
    for r in range(NCORES):
        m = dict(shared)
        m.update(per_core[r])
        in_maps.append(m)
    return in_maps


def run(cfg, inputs, **kw):
    from concourse import bass_utils

    key = (cfg.B, cfg.S, cfg.D, cfg.H, cfg.HD, cfg.FF)
    if key not in _CACHE:
        _CACHE[key] = build(cfg)
    nc = _CACHE[key]
    in_maps = _prep_inputs(cfg, **{k: v for k, v in inputs.items()
                                   if k != "start_pos"})
    res = bass_utils.run_bass_kernel_spmd(
        nc, in_maps, core_ids=list(range(NCORES)), **kw)
    outs = []
    for s in ("x", "y"):
        cols = [res.results[r][f"out_{s}"] for r in range(NCORES)]
        full_t = np.concatenate(cols, axis=1)           # [D, T]
        outs.append(np.ascontiguousarray(full_t.T)
                    .reshape(cfg.B, cfg.S, cfg.D).astype(np.float32))
    return tuple(outs), res


def kernel(**inputs):
    (out_x, out_y), _ = run(FULL, inputs)
    return out_x, out_y
